# revision 14
# baseline (speedup 1.0000x reference)
"""Trainium2 Bass kernel for nn_MenuLoss_7713761264358.

Strategy (data parallel over 8 NeuronCores, 64 batches each):

Every table lookup in the reference collapses to a row gather
data[id, :] because the soft-gaussian weights are exact one-hot
selectors for integer ids (pred ids after round+mask, true ids by
construction).  The previous kernel did those gathers with GPSIMD
ap_gather at ~27.5 ns/index serialized on the 8 Q7 cores (74 us for
21504 indices/core).  This version uses dma_gather: Q7 cores only
*generate* SWDGE descriptors and the 16 parallel SDMA engines execute
the 256-byte row fetches from HBM.  The SWDGE descriptor ring caps a
single call at 1024 indices, so the gather is issued as 21 calls of
1024 round-robined over 4 SWDGE queues (each queue owns its own Q7
core pair and descriptor ring, so generation and ring-reclaim overlap
across queues while the DMA engines stream continuously).

Layout trick: dma_gather writes gathered row i to partition i%128,
block i//128.  Ordering the 21504 per-core lookups as i = 128*j + P
with P = batch for pred (P in 0..63) and P = 64 + batch for true
puts EVERY batch's 168 tokens (j, in day-major order) on a single
partition.  All per-batch / per-day / per-meal sums become free-dim
DVE reductions with zero cross-partition traffic and no PE broadcast
of amounts.  Pred-vs-true combination happens at the very end via two
tiny [128]x[64] matmuls (difference and true-select), and the final
scalar is a ones-vector contraction.

Table rows are padded to 64 fp32 (256 B, the dma_gather minimum):
cols 0..4 = continuous nutrition, cols 5..18 = the 14 raw binary
columns (no packing needed -- the row fetch is free-width).

Host work is layout-only: shard batches, de-interleave id/amount,
replicate the wrapped int16-index source across the 8 Q7 groups, pad
the table, and sum the 8 per-core partial losses.
"""

import numpy as np

import concourse.bass as bass
import concourse.tile as tile
from concourse import bacc, mybir

AF = mybir.ActivationFunctionType
OP = mybir.AluOpType
AX = mybir.AxisListType
F32 = mybir.dt.float32
I16 = mybir.dt.int16

NCORES = 8
BG = 512            # global batch
BL = BG // NCORES   # 64 batches per core
S = 168             # tokens per batch (7 days * 3 meals * 8 foods)
NIDX = 128 * S      # 21504 gather indices per core
W = NIDX // 16      # 1344 idx columns (16-partition wrap)
E = 19              # gathered row width in fp32 (76B descriptors)
EP = 64             # table row pitch in fp32 (256B, the hw pitch requirement)
CALL = 1024         # indices per dma_gather call (SWDGE ring limit)
CB = CALL // 128    # 8 out blocks per call
NCALL = NIDX // CALL  # 21 calls
J0 = 96             # consume split: days 0..3 | days 4..6

MAGIC = 8388608.0   # 2^23: (x + MAGIC) - MAGIC == round-half-even(x)
ZCONST = 3000.0 * 3.0 * S * BL / float(BG)  # constant part of zeros penalty

# cst tile column map
C_D = 0             # [128, 64] pred-minus-true combine matrix
C_T = 64            # [128, 64] true-select matrix
C_W = 128           # [64, FINW] final per-column weights
C_ONE = 152         # all-ones column
C_M222 = 153        # -222.0 (relu bias)
C_M1680 = 154       # -1680.0 (prefs exp bias)
CSTW = 155

# fin tile columns: huber(nut 5 | meal 3 | ing 5) | pref 2 | alrg 7 | var | sta
F_HUB, F_PREF, F_ALRG, F_VAR, F_STA, FINW = 0, 13, 15, 22, 23, 24


def _dma_gather_raw(gp, out_ap, in_ap, idxs_ap, num_idxs, elem_size,
                    elem_step, queue_num):
    """bass.dma_gather minus the `elem_size_bytes % 256` assert, which the
    ucode and NX decode only require for transpose mode.  Non-transpose
    descriptors may be any length; only the source row pitch (elem_step)
    must be a multiple of 256 bytes."""
    stride_bytes = elem_step * 4
    assert stride_bytes % 256 == 0
    _in_ap = gp.lower_ap_dma(in_ap, for_custom_bir_dma=True)
    return gp.add_instruction(
        mybir.InstDMAGatherAnt(
            name=gp.bass.get_next_instruction_name(),
            ins=[
                *_in_ap,
                gp.lower_ap(idxs_ap),
                gp.lower_val_access(gp.to_reg(num_idxs)),
            ],
            outs=[gp.lower_ap(out_ap)],
            transpose=False,
            num_idxs=num_idxs,
            elem_size=elem_size,
            stride_bytes_256=stride_bytes // 256,
            gen_mode=0,
            single_packet=True,
            queue_num=queue_num,
            sbuf_tokens_per_rank=0,
            sbuf_free_dim_per_rank=0,
            sbuf_free_dim_pad_per_rank=0,
            sbuf_byte_offset=0,
        )
    )


def _build(tc, idp, idt, amt, pidp, tabs, cst, out):
    import contextlib

    nc = tc.nc
    from concourse import library_config

    with contextlib.ExitStack() as ctx:
        sb = ctx.enter_context(tc.tile_pool(name="sb", bufs=1))
        ps = ctx.enter_context(tc.tile_pool(name="ps", bufs=1, space="PSUM"))

        # Kick the Q7 IRAM load first so it overlaps the input DMAs, and
        # issue a dummy 128-index gather immediately: the first call to a
        # freshly loaded kernel pays the ~6us IRAM copy, so burn it on a
        # throwaway while the real inputs are still streaming in.
        nc.gpsimd.load_library(library_config.mlp)
        idx0 = sb.tile([128, 8], I16, tag="idx0")
        nc.vector.memset(idx0[:], 0)
        g0 = sb.tile([128, 1, E], F32, tag="g0")
        _dma_gather_raw(nc.gpsimd, g0[:], tabs, idx0[:], 128, E, EP, 0)

        # ---- inputs ----
        idp_s = sb.tile([128, W // 2], F32, tag="idp_s")
        nc.sync.dma_start(out=idp_s[:], in_=idp)
        idt_s = sb.tile([128, W // 2], F32, tag="idt_s")
        nc.sync.dma_start(out=idt_s[:], in_=idt)
        amt_s = sb.tile([128, S], F32, tag="amt_s")
        nc.scalar.dma_start(out=amt_s[:], in_=amt)
        pidp_s = sb.tile([128, 84], F32, tag="pidp_s")
        nc.scalar.dma_start(out=pidp_s[:], in_=pidp)
        cst_s = sb.tile([128, CSTW], F32, tag="cst_s")
        nc.scalar.dma_start(out=cst_s[:], in_=cst)

        # ---- gather indices: [128, 1344] int16, i = 16*s + p wrap ----
        # col s = 8j + a: a<4 -> pred batch 16a + p%16, a>=4 -> true.
        idx = sb.tile([128, W], I16, tag="idx")
        idx_v = idx[:].rearrange("p (j a) -> p j a", a=8)
        kp = sb.tile([128, W // 2], F32, tag="kp")
        nc.vector.tensor_scalar(
            out=kp[:], in0=idp_s[:], scalar1=MAGIC, scalar2=MAGIC,
            op0=OP.add, op1=OP.subtract,
        )
        nc.vector.scalar_tensor_tensor(
            out=idx_v[:, :, 0:4],
            in0=kp[:].rearrange("p (j a) -> p j a", a=4), scalar=222.5,
            in1=kp[:].rearrange("p (j a) -> p j a", a=4),
            op0=OP.is_le, op1=OP.mult,
        )
        nc.vector.tensor_copy(
            out=idx_v[:, :, 4:8],
            in_=idt_s[:].rearrange("p (j a) -> p j a", a=4),
        )

        # ---- gathers: g[P, j, :] = tabs[id(P, j), :] ----
        # 21 calls of 1024 indices (SWDGE ring limit) on queues 0..3.
        g = sb.tile([128, S, E], F32, tag="g")
        for k in range(NCALL):
            _dma_gather_raw(
                nc.gpsimd, g[:, k * CB:(k + 1) * CB, :], tabs,
                idx[:, k * (CALL // 16):(k + 1) * (CALL // 16)],
                CALL, E, EP, queue_num=(k + 1) % 4,
            )

        # ---- per-batch reductions (partition P = (type, batch)) ----
        # One consume chunk per day (24 tokens = 3 gather calls) so the
        # reductions pipeline under the gather stream.
        pr = sb.tile([128, 5 * S], F32, tag="pr")   # amount-weighted cont
        prv = pr[:].rearrange("p (c j) -> p c j", j=S)
        day = sb.tile([128, 7], F32, tag="day")
        nut_h = sb.tile([128, 5 * 7], F32, tag="nut_h")
        nut_hv = nut_h[:].rearrange("p (c k) -> p c k", k=7)
        meal_h = sb.tile([128, 3 * 7], F32, tag="meal_h")
        meal_hv = meal_h[:].rearrange("p (m k) -> p m k", k=7)
        cnt_h = sb.tile([128, 14 * 7], F32, tag="cnt_h")
        cnt_hv = cnt_h[:].rearrange("p (c k) -> p c k", k=7)

        def red(out_ap, in_ap, axis=AX.X):
            nc.vector.tensor_reduce(out=out_ap, in_=in_ap, axis=axis, op=OP.add)

        for k in range(7):
            j0, j1 = 24 * k, 24 * k + 24
            nc.vector.tensor_tensor(
                out=prv[:, :, j0:j1],
                in0=g[:, j0:j1, 0:5].rearrange("p j c -> p c j"),
                in1=amt_s[:, j0:j1].unsqueeze(1).broadcast_to([128, 5, 24]),
                op=OP.mult,
            )
            red(nut_hv[:, :, k], prv[:, :, j0:j1])
            red(
                meal_hv[:, :, k],
                pr[:, j0:j1].rearrange("p (m f) -> p m f", f=8),
            )
            red(day[:, k:k + 1], pr[:, j0:j1])
            red(
                cnt_hv[:, :, k],
                g[:, j0:j1, 5:19].rearrange("p j c -> p c j"),
            )

        # acc0 cols: nut 0:5 | meal 5:8 | counts 8:22
        acc0 = sb.tile([128, 22], F32, tag="acc0")
        red(acc0[:, 0:5], nut_hv[:])
        red(acc0[:, 5:8], meal_hv[:])
        red(acc0[:, 8:22], cnt_hv[:])

        # ---- pred/true combine on PE ----
        diff_ps = ps.tile([64, 22], F32, tag="diff_ps")
        nc.tensor.matmul(
            diff_ps[:], cst_s[:, C_D:C_D + 64], acc0[:], start=True, stop=True
        )
        true_ps = ps.tile([64, 22], F32, tag="true_ps")
        nc.tensor.matmul(
            true_ps[:], cst_s[:, C_T:C_T + 64], acc0[:], start=True, stop=True
        )

        fin = sb.tile([64, FINW], F32, tag="fin")

        # ---- huber terms: nut/meal at scale 1/700, ingredients at 1 ----
        ha = sb.tile([64, 13], F32, tag="ha")
        nc.scalar.activation(
            out=ha[:, 0:8], in_=diff_ps[:, 0:8], func=AF.Abs, scale=1.0 / 700.0
        )
        nc.scalar.activation(
            out=ha[:, 8:13], in_=diff_ps[:, 17:22], func=AF.Abs, scale=1.0
        )
        hm = sb.tile([64, 13], F32, tag="hm")
        nc.vector.tensor_scalar(
            out=hm[:], in0=ha[:], scalar1=1.0, scalar2=None, op0=OP.min
        )
        ht = sb.tile([64, 13], F32, tag="ht")
        nc.vector.scalar_tensor_tensor(
            out=ht[:], in0=hm[:], scalar=-0.5, in1=ha[:], op0=OP.mult, op1=OP.add
        )
        nc.vector.tensor_tensor(
            out=fin[:, F_HUB:F_PREF], in0=hm[:], in1=ht[:], op=OP.mult
        )

        # ---- prefs: exp(10*cnt_t - 1680) * (168 - cnt_p)^2 ----
        e1 = sb.tile([64, 2], F32, tag="e1")
        nc.scalar.activation(
            out=e1[:], in_=true_ps[:, 8:10], func=AF.Exp, scale=10.0,
            bias=cst_s[0:64, C_M1680:C_M1680 + 1],
        )
        p1 = sb.tile([64, 2], F32, tag="p1")
        nc.vector.tensor_scalar(
            out=p1[:], in0=acc0[0:64, 8:10], scalar1=-1.0, scalar2=168.0,
            op0=OP.mult, op1=OP.add,
        )
        q1 = sb.tile([64, 2], F32, tag="q1")
        nc.scalar.activation(out=q1[:], in_=p1[:], func=AF.Square)
        nc.vector.tensor_tensor(
            out=fin[:, F_PREF:F_ALRG], in0=e1[:], in1=q1[:], op=OP.mult
        )

        # ---- allergens: exp(-10*cnt_t) * cnt_p^2 ----
        e2 = sb.tile([64, 7], F32, tag="e2")
        nc.scalar.activation(
            out=e2[:], in_=true_ps[:, 10:17], func=AF.Exp, scale=-10.0
        )
        q2 = sb.tile([64, 7], F32, tag="q2")
        nc.scalar.activation(out=q2[:], in_=acc0[0:64, 10:17], func=AF.Square)
        nc.vector.tensor_tensor(
            out=fin[:, F_ALRG:F_VAR], in0=e2[:], in1=q2[:], op=OP.mult
        )

        # ---- day-calorie variance (pred rows): var = S2/7 - (S1/700)^2 ----
        s1 = sb.tile([64, 1], F32, tag="s1")
        red(s1[:], day[0:64, :])
        sq = sb.tile([64, 7], F32, tag="sq")
        nc.scalar.activation(out=sq[:], in_=day[0:64, :], func=AF.Square, scale=0.01)
        s2 = sb.tile([64, 1], F32, tag="s2")
        red(s2[:], sq[:])
        mu2 = sb.tile([64, 1], F32, tag="mu2")
        nc.vector.scalar_tensor_tensor(
            out=mu2[:], in0=s1[:], scalar=1.0 / 490000.0, in1=s1[:],
            op0=OP.mult, op1=OP.mult,
        )
        nc.vector.scalar_tensor_tensor(
            out=fin[:, F_VAR:F_STA], in0=s2[:], scalar=1.0 / 7.0, in1=mu2[:],
            op0=OP.mult, op1=OP.subtract,
        )

        # ---- tanh / relu penalties ----
        tha = sb.tile([64, S], F32, tag="tha")
        nc.scalar.activation(
            out=tha[:], in_=amt_s[0:64, :], func=AF.Tanh, scale=2.0,
            accum_out=fin[:, F_STA:F_STA + 1],
        )
        th1 = sb.tile([128, 84], F32, tag="th1")
        st1 = sb.tile([128, 1], F32, tag="st1")
        nc.scalar.activation(
            out=th1[:], in_=pidp_s[:], func=AF.Tanh, scale=2.0, accum_out=st1[:]
        )
        rl1 = sb.tile([128, 84], F32, tag="rl1")
        srel = sb.tile([128, 1], F32, tag="srel")
        nc.scalar.activation(
            out=rl1[:], in_=pidp_s[:], func=AF.Relu, scale=1.0,
            bias=cst_s[:, C_M222:C_M222 + 1], accum_out=srel[:],
        )

        # ---- weight, contract, and emit ----
        wacc = sb.tile([64, FINW], F32, tag="wacc")
        nc.vector.tensor_tensor(
            out=wacc[:], in0=fin[:], in1=cst_s[0:64, C_W:C_W + FINW], op=OP.mult
        )
        acc2 = sb.tile([128, 2], F32, tag="acc2")
        nc.vector.tensor_scalar_mul(
            out=acc2[:, 0:1], in0=st1[:], scalar1=-2.0 * 3000.0 / float(BG)
        )
        nc.vector.tensor_scalar_mul(
            out=acc2[:, 1:2], in0=srel[:], scalar1=1.0 / float(BG)
        )
        fps = ps.tile([1, FINW + 2], F32, tag="fps")
        nc.tensor.matmul(
            fps[:, 0:FINW], cst_s[0:64, C_ONE:C_ONE + 1], wacc[:],
            start=True, stop=True,
        )
        nc.tensor.matmul(
            fps[:, FINW:FINW + 2], cst_s[:, C_ONE:C_ONE + 1], acc2[:],
            start=True, stop=True,
        )
        loss_t = sb.tile([1, 1], F32, tag="loss_t")
        nc.vector.tensor_reduce(out=loss_t[:], in_=fps[:], axis=AX.X, op=OP.add)
        lossf = sb.tile([1, 1], F32, tag="lossf")
        nc.vector.tensor_scalar_add(out=lossf[:], in0=loss_t[:], scalar1=ZCONST)
        nc.sync.dma_start(out=out, in_=lossf[:])


def build_program():
    nc = bacc.Bacc(
        "TRN2", target_bir_lowering=False, num_devices=NCORES,
        num_swdge_queues=4,
    )
    idp = nc.dram_tensor("idp", [128, W // 2], F32, kind="ExternalInput")
    idt = nc.dram_tensor("idt", [128, W // 2], F32, kind="ExternalInput")
    amt = nc.dram_tensor("amt", [128, S], F32, kind="ExternalInput")
    pidp = nc.dram_tensor("pidp", [128, 84], F32, kind="ExternalInput")
    tabs = nc.dram_tensor("tabs", [223, EP], F32, kind="ExternalInput")
    cst = nc.dram_tensor("cst", [128, CSTW], F32, kind="ExternalInput")
    out = nc.dram_tensor("o", [1, 1], F32, kind="ExternalOutput")
    with tile.TileContext(nc) as tc:
        _build(
            tc, idp.ap(), idt.ap(), amt.ap(), pidp.ap(),
            tabs[:, 0:E], cst.ap(), out.ap(),
        )
    nc.compile()
    return nc


def make_const_inputs(data):
    """Constant tables shared by all cores."""
    data = np.asarray(data, dtype=np.float32)
    tabs = np.zeros((223, EP), np.float32)
    tabs[:, 0:19] = data

    cst = np.zeros((128, CSTW), np.float32)
    b = np.arange(64)
    cst[b, C_D + b] = 1.0
    cst[64 + b, C_D + b] = -1.0
    cst[64 + b, C_T + b] = 1.0
    w_hub = 1.0 / (100.0 * BG)
    w_pa = 100.0 / BG
    wgt = np.zeros(FINW, np.float32)
    wgt[F_HUB:F_PREF] = w_hub
    wgt[F_PREF:F_ALRG] = w_pa
    wgt[F_ALRG:F_VAR] = w_pa
    wgt[F_VAR] = 1.0 / BG
    wgt[F_STA] = -3000.0 / BG
    cst[0:64, C_W:C_W + FINW] = wgt
    cst[:, C_ONE] = 1.0
    cst[:, C_M222] = -222.0
    cst[:, C_M1680] = -1680.0
    return tabs, cst


def _wrap_ids(ids_2d):
    """[64, 168] per-batch ids -> [128, 672] replicated idx-source layout:
    out[p, 4j + a] = ids_2d[16a + p % 16, j]."""
    arr = np.ascontiguousarray(ids_2d, dtype=np.float32).reshape(4, 16, S)
    arr = arr.transpose(1, 2, 0).reshape(16, 4 * S)
    return np.tile(arr, (8, 1)).copy()


def make_in_maps(y_pred, y, data):
    y_pred = np.asarray(y_pred, dtype=np.float32)
    y = np.asarray(y, dtype=np.float32)
    tabs, cst = make_const_inputs(data)
    in_maps = []
    for core in range(NCORES):
        sl = slice(core * BL, (core + 1) * BL)
        pid = y_pred[sl, ..., 0].reshape(BL, S)
        pamt = y_pred[sl, ..., 1].reshape(BL, S)
        tid = y[sl, ..., 0].reshape(BL, S)
        tamt = y[sl, ..., 1].reshape(BL, S)
        amt = np.concatenate([pamt, tamt], axis=0)  # [128, 168]
        in_maps.append({
            "idp": _wrap_ids(pid),
            "idt": _wrap_ids(tid),
            "amt": np.ascontiguousarray(amt, dtype=np.float32),
            "pidp": np.ascontiguousarray(pid.reshape(128, 84), dtype=np.float32),
            "tabs": tabs, "cst": cst,
        })
    return in_maps


_NC_CACHE = None


def _get_nc():
    global _NC_CACHE
    if _NC_CACHE is None:
        _NC_CACHE = build_program()
    return _NC_CACHE


def run_on_hw(y_pred, y, data, **kwargs):
    from concourse.bass_utils import run_bass_kernel_spmd

    nc = _get_nc()
    in_maps = make_in_maps(y_pred, y, data)
    res = run_bass_kernel_spmd(
        nc, in_maps, core_ids=list(range(NCORES)), **kwargs
    )
    parts = [r["o"][0, 0] for r in res.results]
    return np.float32(np.sum(np.asarray(parts, dtype=np.float32))), res


def kernel(y_pred, y, data):
    return run_on_hw(y_pred, y, data)[0]


# revision 16
# speedup vs baseline: 1.1302x; 1.1302x over previous
"""Trainium2 Bass kernel for nn_MenuLoss_7713761264358.

Strategy (data parallel over 8 NeuronCores, 64 batches each):

Every table lookup in the reference collapses to a row gather
data[id, :] because the soft-gaussian weights are exact one-hot
selectors for integer ids (pred ids after round+mask, true ids by
construction).  The previous kernel did those gathers with GPSIMD
ap_gather at ~27.5 ns/index serialized on the 8 Q7 cores (74 us for
21504 indices/core).  This version uses dma_gather: Q7 cores only
*generate* SWDGE descriptors and the 16 parallel SDMA engines execute
the 256-byte row fetches from HBM.  The SWDGE descriptor ring caps a
single call at 1024 indices, so the gather is issued as 21 calls of
1024 round-robined over 4 SWDGE queues (each queue owns its own Q7
core pair and descriptor ring, so generation and ring-reclaim overlap
across queues while the DMA engines stream continuously).

Layout trick: dma_gather writes gathered row i to partition i%128,
block i//128.  Ordering the 21504 per-core lookups as i = 128*j + P
with P = batch for pred (P in 0..63) and P = 64 + batch for true
puts EVERY batch's 168 tokens (j, in day-major order) on a single
partition.  All per-batch / per-day / per-meal sums become free-dim
DVE reductions with zero cross-partition traffic and no PE broadcast
of amounts.  Pred-vs-true combination happens at the very end via two
tiny [128]x[64] matmuls (difference and true-select), and the final
scalar is a ones-vector contraction.

Table rows are padded to 64 fp32 (256 B, the dma_gather minimum):
cols 0..4 = continuous nutrition, cols 5..18 = the 14 raw binary
columns (no packing needed -- the row fetch is free-width).

Host work is layout-only: shard batches, de-interleave id/amount,
replicate the wrapped int16-index source across the 8 Q7 groups, pad
the table, and sum the 8 per-core partial losses.
"""

import numpy as np

import concourse.bass as bass
import concourse.tile as tile
from concourse import bacc, mybir

AF = mybir.ActivationFunctionType
OP = mybir.AluOpType
AX = mybir.AxisListType
F32 = mybir.dt.float32
I16 = mybir.dt.int16

NCORES = 8
BG = 512            # global batch
BL = BG // NCORES   # 64 batches per core
S = 168             # tokens per batch (7 days * 3 meals * 8 foods)
NIDX = 128 * S      # 21504 gather indices per core
W = NIDX // 16      # 1344 idx columns (16-partition wrap)
E = 19              # gathered row width in fp32 (76B descriptors)
EP = 64             # table row pitch in fp32 (256B, the hw pitch requirement)
CALL = 1024         # indices per dma_gather call (SWDGE ring limit)
CB = CALL // 128    # 8 out blocks per call
NCALL = NIDX // CALL  # 21 calls
J0 = 96             # consume split: days 0..3 | days 4..6

MAGIC = 8388608.0   # 2^23: (x + MAGIC) - MAGIC == round-half-even(x)
ZCONST = 3000.0 * 3.0 * S * BL / float(BG)  # constant part of zeros penalty

# cst tile column map
C_D = 0             # [128, 64] pred-minus-true combine matrix
C_T = 64            # [128, 64] true-select matrix
C_W = 128           # [64, FINW] final per-column weights
C_ONE = 152         # all-ones column
C_M222 = 153        # -222.0 (relu bias)
C_M1680 = 154       # -1680.0 (prefs exp bias)
CSTW = 155

# fin tile columns: huber(nut 5 | meal 3 | ing 5) | pref 2 | alrg 7 | var | sta
F_HUB, F_PREF, F_ALRG, F_VAR, F_STA, FINW = 0, 13, 15, 22, 23, 24


def _dma_gather_raw(gp, out_ap, in_ap, idxs_ap, num_idxs, elem_size,
                    elem_step, queue_num):
    """bass.dma_gather minus the `elem_size_bytes % 256` assert, which the
    ucode and NX decode only require for transpose mode.  Non-transpose
    descriptors may be any length; only the source row pitch (elem_step)
    must be a multiple of 256 bytes."""
    stride_bytes = elem_step * 4
    assert stride_bytes % 256 == 0
    _in_ap = gp.lower_ap_dma(in_ap, for_custom_bir_dma=True)
    return gp.add_instruction(
        mybir.InstDMAGatherAnt(
            name=gp.bass.get_next_instruction_name(),
            ins=[
                *_in_ap,
                gp.lower_ap(idxs_ap),
                gp.lower_val_access(gp.to_reg(num_idxs)),
            ],
            outs=[gp.lower_ap(out_ap)],
            transpose=False,
            num_idxs=num_idxs,
            elem_size=elem_size,
            stride_bytes_256=stride_bytes // 256,
            gen_mode=0,
            single_packet=True,
            queue_num=queue_num,
            sbuf_tokens_per_rank=0,
            sbuf_free_dim_per_rank=0,
            sbuf_free_dim_pad_per_rank=0,
            sbuf_byte_offset=0,
        )
    )


def _build(tc, idp, idt, amt, pidp, tabs, cst, out):
    import contextlib

    nc = tc.nc
    from concourse import library_config

    with contextlib.ExitStack() as ctx:
        sb = ctx.enter_context(tc.tile_pool(name="sb", bufs=1))
        ps = ctx.enter_context(tc.tile_pool(name="ps", bufs=1, space="PSUM"))

        # Kick the Q7 IRAM load first so it overlaps the input DMAs.
        nc.gpsimd.load_library(library_config.mlp)

        # ---- inputs ----
        idp_s = sb.tile([128, W // 2], F32, tag="idp_s")
        nc.sync.dma_start(out=idp_s[:], in_=idp)
        idt_s = sb.tile([128, W // 2], F32, tag="idt_s")
        nc.sync.dma_start(out=idt_s[:], in_=idt)
        amt_s = sb.tile([128, S], F32, tag="amt_s")
        nc.scalar.dma_start(out=amt_s[:], in_=amt)
        pidp_s = sb.tile([128, 84], F32, tag="pidp_s")
        nc.scalar.dma_start(out=pidp_s[:], in_=pidp)
        cst_s = sb.tile([128, CSTW], F32, tag="cst_s")
        nc.scalar.dma_start(out=cst_s[:], in_=cst)

        # ---- gather indices: [128, 1344] int16, i = 16*s + p wrap ----
        # col s = 8j + a: a<4 -> pred batch 16a + p%16, a>=4 -> true.
        idx = sb.tile([128, W], I16, tag="idx")
        idx_v = idx[:].rearrange("p (j a) -> p j a", a=8)
        kp = sb.tile([128, W // 2], F32, tag="kp")
        nc.vector.tensor_scalar(
            out=kp[:], in0=idp_s[:], scalar1=MAGIC, scalar2=MAGIC,
            op0=OP.add, op1=OP.subtract,
        )
        nc.vector.scalar_tensor_tensor(
            out=idx_v[:, :, 0:4],
            in0=kp[:].rearrange("p (j a) -> p j a", a=4), scalar=222.5,
            in1=kp[:].rearrange("p (j a) -> p j a", a=4),
            op0=OP.is_le, op1=OP.mult,
        )
        nc.vector.tensor_copy(
            out=idx_v[:, :, 4:8],
            in_=idt_s[:].rearrange("p (j a) -> p j a", a=4),
        )

        # ---- gathers: g[P, j, :] = tabs[id(P, j), :] ----
        # 21 calls of 1024 indices (SWDGE ring limit) on queues 0..3.
        g = sb.tile([128, S, E], F32, tag="g")
        for k in range(NCALL):
            _dma_gather_raw(
                nc.gpsimd, g[:, k * CB:(k + 1) * CB, :], tabs,
                idx[:, k * (CALL // 16):(k + 1) * (CALL // 16)],
                CALL, E, EP, queue_num=k % 4,
            )

        # ---- per-batch reductions (partition P = (type, batch)) ----
        # Two consume chunks, split 6 days | 1 day: the big chunk overlaps
        # the gather stream, the tiny tail chunk follows the last call.
        pr = sb.tile([128, 5 * S], F32, tag="pr")   # amount-weighted cont
        prv = pr[:].rearrange("p (c j) -> p c j", j=S)
        day = sb.tile([128, 7], F32, tag="day")
        nut_h = sb.tile([128, 5 * 2], F32, tag="nut_h")
        meal_h = sb.tile([128, 3 * 2], F32, tag="meal_h")
        cnt_h = sb.tile([128, 14 * 2], F32, tag="cnt_h")

        def red(out_ap, in_ap, axis=AX.X):
            nc.vector.tensor_reduce(out=out_ap, in_=in_ap, axis=axis, op=OP.add)

        for h, (j0, j1) in enumerate(((0, 144), (144, S))):
            w = j1 - j0
            nc.vector.tensor_tensor(
                out=prv[:, :, j0:j1],
                in0=g[:, j0:j1, 0:5].rearrange("p j c -> p c j"),
                in1=amt_s[:, j0:j1].unsqueeze(1).broadcast_to([128, 5, w]),
                op=OP.mult,
            )
            red(nut_h[:, 5 * h:5 * h + 5], prv[:, :, j0:j1])
            red(
                meal_h[:, 3 * h:3 * h + 3],
                pr[:, j0:j1].rearrange("p (d m f) -> p m d f", m=3, f=8),
                axis=AX.XY,
            )
            red(
                day[:, j0 // 24:j1 // 24],
                pr[:, j0:j1].rearrange("p (d u) -> p d u", u=24),
            )
            red(
                cnt_h[:, 14 * h:14 * h + 14],
                g[:, j0:j1, 5:19].rearrange("p j c -> p c j"),
            )

        # acc0 cols: nut 0:5 | meal 5:8 | counts 8:22
        acc0 = sb.tile([128, 22], F32, tag="acc0")
        for (dst0, src, n) in ((0, nut_h, 5), (5, meal_h, 3), (8, cnt_h, 14)):
            nc.vector.tensor_tensor(
                out=acc0[:, dst0:dst0 + n], in0=src[:, 0:n], in1=src[:, n:2 * n],
                op=OP.add,
            )

        # ---- pred/true combine on PE ----
        diff_ps = ps.tile([64, 22], F32, tag="diff_ps")
        nc.tensor.matmul(
            diff_ps[:], cst_s[:, C_D:C_D + 64], acc0[:], start=True, stop=True
        )
        true_ps = ps.tile([64, 22], F32, tag="true_ps")
        nc.tensor.matmul(
            true_ps[:], cst_s[:, C_T:C_T + 64], acc0[:], start=True, stop=True
        )

        fin = sb.tile([64, FINW], F32, tag="fin")

        # ---- huber terms: nut/meal at scale 1/700, ingredients at 1 ----
        ha = sb.tile([64, 13], F32, tag="ha")
        nc.scalar.activation(
            out=ha[:, 0:8], in_=diff_ps[:, 0:8], func=AF.Abs, scale=1.0 / 700.0
        )
        nc.scalar.activation(
            out=ha[:, 8:13], in_=diff_ps[:, 17:22], func=AF.Abs, scale=1.0
        )
        hm = sb.tile([64, 13], F32, tag="hm")
        nc.vector.tensor_scalar(
            out=hm[:], in0=ha[:], scalar1=1.0, scalar2=None, op0=OP.min
        )
        ht = sb.tile([64, 13], F32, tag="ht")
        nc.vector.scalar_tensor_tensor(
            out=ht[:], in0=hm[:], scalar=-0.5, in1=ha[:], op0=OP.mult, op1=OP.add
        )
        nc.vector.tensor_tensor(
            out=fin[:, F_HUB:F_PREF], in0=hm[:], in1=ht[:], op=OP.mult
        )

        # ---- prefs: exp(10*cnt_t - 1680) * (168 - cnt_p)^2 ----
        e1 = sb.tile([64, 2], F32, tag="e1")
        nc.scalar.activation(
            out=e1[:], in_=true_ps[:, 8:10], func=AF.Exp, scale=10.0,
            bias=cst_s[0:64, C_M1680:C_M1680 + 1],
        )
        p1 = sb.tile([64, 2], F32, tag="p1")
        nc.vector.tensor_scalar(
            out=p1[:], in0=acc0[0:64, 8:10], scalar1=-1.0, scalar2=168.0,
            op0=OP.mult, op1=OP.add,
        )
        q1 = sb.tile([64, 2], F32, tag="q1")
        nc.scalar.activation(out=q1[:], in_=p1[:], func=AF.Square)
        nc.vector.tensor_tensor(
            out=fin[:, F_PREF:F_ALRG], in0=e1[:], in1=q1[:], op=OP.mult
        )

        # ---- allergens: exp(-10*cnt_t) * cnt_p^2 ----
        e2 = sb.tile([64, 7], F32, tag="e2")
        nc.scalar.activation(
            out=e2[:], in_=true_ps[:, 10:17], func=AF.Exp, scale=-10.0
        )
        q2 = sb.tile([64, 7], F32, tag="q2")
        nc.scalar.activation(out=q2[:], in_=acc0[0:64, 10:17], func=AF.Square)
        nc.vector.tensor_tensor(
            out=fin[:, F_ALRG:F_VAR], in0=e2[:], in1=q2[:], op=OP.mult
        )

        # ---- day-calorie variance (pred rows): var = S2/7 - (S1/700)^2 ----
        s1 = sb.tile([64, 1], F32, tag="s1")
        red(s1[:], day[0:64, :])
        sq = sb.tile([64, 7], F32, tag="sq")
        nc.scalar.activation(out=sq[:], in_=day[0:64, :], func=AF.Square, scale=0.01)
        s2 = sb.tile([64, 1], F32, tag="s2")
        red(s2[:], sq[:])
        mu2 = sb.tile([64, 1], F32, tag="mu2")
        nc.vector.scalar_tensor_tensor(
            out=mu2[:], in0=s1[:], scalar=1.0 / 490000.0, in1=s1[:],
            op0=OP.mult, op1=OP.mult,
        )
        nc.vector.scalar_tensor_tensor(
            out=fin[:, F_VAR:F_STA], in0=s2[:], scalar=1.0 / 7.0, in1=mu2[:],
            op0=OP.mult, op1=OP.subtract,
        )

        # ---- tanh / relu penalties ----
        tha = sb.tile([64, S], F32, tag="tha")
        nc.scalar.activation(
            out=tha[:], in_=amt_s[0:64, :], func=AF.Tanh, scale=2.0,
            accum_out=fin[:, F_STA:F_STA + 1],
        )
        th1 = sb.tile([128, 84], F32, tag="th1")
        st1 = sb.tile([128, 1], F32, tag="st1")
        nc.scalar.activation(
            out=th1[:], in_=pidp_s[:], func=AF.Tanh, scale=2.0, accum_out=st1[:]
        )
        rl1 = sb.tile([128, 84], F32, tag="rl1")
        srel = sb.tile([128, 1], F32, tag="srel")
        nc.scalar.activation(
            out=rl1[:], in_=pidp_s[:], func=AF.Relu, scale=1.0,
            bias=cst_s[:, C_M222:C_M222 + 1], accum_out=srel[:],
        )

        # ---- weight, contract, and emit ----
        wacc = sb.tile([64, FINW], F32, tag="wacc")
        nc.vector.tensor_tensor(
            out=wacc[:], in0=fin[:], in1=cst_s[0:64, C_W:C_W + FINW], op=OP.mult
        )
        acc2 = sb.tile([128, 2], F32, tag="acc2")
        nc.vector.tensor_scalar_mul(
            out=acc2[:, 0:1], in0=st1[:], scalar1=-2.0 * 3000.0 / float(BG)
        )
        nc.vector.tensor_scalar_mul(
            out=acc2[:, 1:2], in0=srel[:], scalar1=1.0 / float(BG)
        )
        fps = ps.tile([1, FINW + 2], F32, tag="fps")
        nc.tensor.matmul(
            fps[:, 0:FINW], cst_s[0:64, C_ONE:C_ONE + 1], wacc[:],
            start=True, stop=True,
        )
        nc.tensor.matmul(
            fps[:, FINW:FINW + 2], cst_s[:, C_ONE:C_ONE + 1], acc2[:],
            start=True, stop=True,
        )
        loss_t = sb.tile([1, 1], F32, tag="loss_t")
        nc.vector.tensor_reduce(out=loss_t[:], in_=fps[:], axis=AX.X, op=OP.add)
        lossf = sb.tile([1, 1], F32, tag="lossf")
        nc.vector.tensor_scalar_add(out=lossf[:], in0=loss_t[:], scalar1=ZCONST)
        nc.sync.dma_start(out=out, in_=lossf[:])


def build_program():
    nc = bacc.Bacc(
        "TRN2", target_bir_lowering=False, num_devices=NCORES,
        num_swdge_queues=4,
    )
    idp = nc.dram_tensor("idp", [128, W // 2], F32, kind="ExternalInput")
    idt = nc.dram_tensor("idt", [128, W // 2], F32, kind="ExternalInput")
    amt = nc.dram_tensor("amt", [128, S], F32, kind="ExternalInput")
    pidp = nc.dram_tensor("pidp", [128, 84], F32, kind="ExternalInput")
    tabs = nc.dram_tensor("tabs", [223, EP], F32, kind="ExternalInput")
    cst = nc.dram_tensor("cst", [128, CSTW], F32, kind="ExternalInput")
    out = nc.dram_tensor("o", [1, 1], F32, kind="ExternalOutput")
    with tile.TileContext(nc) as tc:
        _build(
            tc, idp.ap(), idt.ap(), amt.ap(), pidp.ap(),
            tabs[:, 0:E], cst.ap(), out.ap(),
        )
    nc.compile()
    return nc


def make_const_inputs(data):
    """Constant tables shared by all cores."""
    data = np.asarray(data, dtype=np.float32)
    tabs = np.zeros((223, EP), np.float32)
    tabs[:, 0:19] = data

    cst = np.zeros((128, CSTW), np.float32)
    b = np.arange(64)
    cst[b, C_D + b] = 1.0
    cst[64 + b, C_D + b] = -1.0
    cst[64 + b, C_T + b] = 1.0
    w_hub = 1.0 / (100.0 * BG)
    w_pa = 100.0 / BG
    wgt = np.zeros(FINW, np.float32)
    wgt[F_HUB:F_PREF] = w_hub
    wgt[F_PREF:F_ALRG] = w_pa
    wgt[F_ALRG:F_VAR] = w_pa
    wgt[F_VAR] = 1.0 / BG
    wgt[F_STA] = -3000.0 / BG
    cst[0:64, C_W:C_W + FINW] = wgt
    cst[:, C_ONE] = 1.0
    cst[:, C_M222] = -222.0
    cst[:, C_M1680] = -1680.0
    return tabs, cst


def _wrap_ids(ids_2d):
    """[64, 168] per-batch ids -> [128, 672] replicated idx-source layout:
    out[p, 4j + a] = ids_2d[16a + p % 16, j]."""
    arr = np.ascontiguousarray(ids_2d, dtype=np.float32).reshape(4, 16, S)
    arr = arr.transpose(1, 2, 0).reshape(16, 4 * S)
    return np.tile(arr, (8, 1)).copy()


def make_in_maps(y_pred, y, data):
    y_pred = np.asarray(y_pred, dtype=np.float32)
    y = np.asarray(y, dtype=np.float32)
    tabs, cst = make_const_inputs(data)
    in_maps = []
    for core in range(NCORES):
        sl = slice(core * BL, (core + 1) * BL)
        pid = y_pred[sl, ..., 0].reshape(BL, S)
        pamt = y_pred[sl, ..., 1].reshape(BL, S)
        tid = y[sl, ..., 0].reshape(BL, S)
        tamt = y[sl, ..., 1].reshape(BL, S)
        amt = np.concatenate([pamt, tamt], axis=0)  # [128, 168]
        in_maps.append({
            "idp": _wrap_ids(pid),
            "idt": _wrap_ids(tid),
            "amt": np.ascontiguousarray(amt, dtype=np.float32),
            "pidp": np.ascontiguousarray(pid.reshape(128, 84), dtype=np.float32),
            "tabs": tabs, "cst": cst,
        })
    return in_maps


_NC_CACHE = None


def _get_nc():
    global _NC_CACHE
    if _NC_CACHE is None:
        _NC_CACHE = build_program()
    return _NC_CACHE


def run_on_hw(y_pred, y, data, **kwargs):
    from concourse.bass_utils import run_bass_kernel_spmd

    nc = _get_nc()
    in_maps = make_in_maps(y_pred, y, data)
    res = run_bass_kernel_spmd(
        nc, in_maps, core_ids=list(range(NCORES)), **kwargs
    )
    parts = [r["o"][0, 0] for r in res.results]
    return np.float32(np.sum(np.asarray(parts, dtype=np.float32))), res


def kernel(y_pred, y, data):
    return run_on_hw(y_pred, y, data)[0]
